# revision 18
# baseline (speedup 1.0000x reference)
"""BiMamba block kernel for 8 Trainium2 NeuronCores.

Sharding: core = 2*sample + direction (4 samples x 2 scan directions).
Each core runs the full mamba for its (sample, direction): input/gate
projections (PE), causal depthwise conv (PE diag-matmul), dt softplus
(ACT), then 16 selective-scan states via the DVE tensor_tensor_scan
instruction, with exp(dt*A) on ACT, dt*x*B on GPSIMD, C*h on DVE and
the sum over states accumulated in PSUM by identity matmuls.  The tail
(3x3 conv partials, BatchNorm batch stats, residual, LeakyReLU) uses a
pair AllReduce plus an 8-core stats AllReduce.
"""
import os
import sys

for _p in ("/opt/trn_rl_repo", "/root/.axon_site/_ro/trn_rl_repo"):
    if os.path.isdir(_p):
        if _p not in sys.path:
            sys.path.insert(0, _p)
        break

import ml_dtypes
import numpy as np

# The agent image's antenv lacks axon_hooks; inject it so trace=True can
# capture NTFF profiles (used by test.py for HW timing, not for grading).
try:
    import antenv.axon_hooks  # noqa: F401
except ImportError:
    try:
        import types as _types

        from trn_agent_boot.trn_boot import _ntff_profile_via_ctypes

        _hook = _ntff_profile_via_ctypes("/opt/axon/libaxon_pjrt.so")
        _m = _types.ModuleType("antenv.axon_hooks")
        _m.get_axon_ntff_profile_hook = lambda: _hook
        _m.set_axon_ntff_profile_hook = lambda h: None
        sys.modules["antenv.axon_hooks"] = _m
    except Exception:
        pass

import concourse.bass as bass
import concourse.mybir as mybir
from concourse import bacc
from concourse import bass_utils
from concourse.masks import make_identity
from concourse.tile import TileContext

F32 = mybir.dt.float32
BF16 = mybir.dt.bfloat16
AF = mybir.ActivationFunctionType
OP = mybir.AluOpType

B, C, H, W = 4, 64, 64, 64
L = H * W          # 4096
DI = 128           # d_inner
DS = 16            # d_state
DTR = 4            # dt_rank
DCONV = 4
NCORE = 8
CH = 512           # matmul free-dim chunk
NCH = L // CH      # 8
HALF = L // 2      # scan chunk length
NHALF = 2

# config switches
BCAST_BF16 = True      # broadcast B/C rows in bf16
G_BF16 = True          # state contributions in bf16 (PE accumulates fp32)
CONV3_BF16 = True      # 3x3 conv in bf16
DBX_BF16 = True        # dt*x*B products and scan output h in bf16
G_ON_GP = 2            # how many of every 8 G-muls go to GPSIMD


def _build():
    nc = bacc.Bacc(target_bir_lowering=False, debug=False, num_devices=NCORE)

    def din(name, shape, dtype=F32):
        return nc.dram_tensor(name, shape, dtype, kind="ExternalInput")

    x_loc = din("x_loc", [C, L])
    in_wT = din("in_wT", [128, 2 * DI])       # padded K (rows 64:128 zero)
    conv1_w = din("conv1_w", [DI, DCONV])
    conv1_b = din("conv1_b", [DI, 1])
    bigproj_T = din("bigproj_T", [DI, DI])    # (dt_w @ xproj_w[:4]).T
    bc_wT = din("bc_wT", [DI, 2 * DS])        # xproj_w[4:36].T
    dt_b = din("dt_b", [DI, 1])
    A_mat = din("A_mat", [DI, DS])
    Dvec = din("Dvec", [DI, 1])
    out_wT = din("out_wT", [DI, C])
    c3dt = BF16 if CONV3_BF16 else F32
    conv3_w = din("conv3_w", [128, 9 * C], c3dt)  # tap-major lhsT, padded K
    conv3_b = din("conv3_b", [C, 1])
    res_wT = din("res_wT", [128, C])          # padded K
    res_b = din("res_b", [C, 1])
    bn_g = din("bn_g", [C, 1])
    bn_b = din("bn_b", [C, 1])

    out_d = nc.dram_tensor("out", [C, L], F32, kind="ExternalOutput")

    gdt = BF16 if G_BF16 else F32
    bdt = BF16 if BCAST_BF16 else F32

    with TileContext(nc) as tc:
        with tc.tile_pool(name="pers", bufs=1) as pers:
            # ---- small persistent tiles (params) ----
            p_in_wT = pers.tile([128, 2 * DI], F32)
            p_c1w = pers.tile([DI, DCONV], F32)
            p_c1b = pers.tile([DI, 1], F32)
            p_bigT = pers.tile([DI, DI], F32)
            p_bcwT = pers.tile([DI, 2 * DS], F32)
            p_dtb = pers.tile([DI, 1], F32)
            p_A = pers.tile([DI, DS], F32)
            p_D = pers.tile([DI, 1], F32)
            p_owT = pers.tile([DI, C], F32)
            p_c3w = pers.tile([128, 9 * C], c3dt)
            p_c3b = pers.tile([C, 1], F32)
            p_rwT = pers.tile([128, C], F32)
            p_rb = pers.tile([C, 1], F32)
            p_bng = pers.tile([C, 1], F32)
            p_bnb = pers.tile([C, 1], F32)
            for t, d in ((p_in_wT, in_wT), (p_c1w, conv1_w), (p_c1b, conv1_b),
                         (p_bigT, bigproj_T), (p_bcwT, bc_wT), (p_dtb, dt_b),
                         (p_A, A_mat), (p_D, Dvec), (p_owT, out_wT),
                         (p_c3w, conv3_w), (p_c3b, conv3_b), (p_rwT, res_wT),
                         (p_rb, res_b), (p_bng, bn_g), (p_bnb, bn_b)):
                nc.sync.dma_start(t[:], d[:])

            ident = pers.tile([128, 128], F32)
            make_identity(nc, ident[:])
            ident_g = pers.tile([128, 128], gdt)
            nc.vector.tensor_copy(ident_g[:], ident[:])
            diag_d = pers.tile([128, 128], F32)
            nc.vector.tensor_scalar_mul(diag_d[:], ident[:], p_D[:, 0:1])
            diag_c1 = [pers.tile([128, 128], F32, tag=f"dgc{k}", name=f"dgc{k}")
                       for k in range(DCONV)]
            for k in range(DCONV):
                nc.vector.tensor_scalar_mul(diag_c1[k][:], ident[:],
                                            p_c1w[:, k:k + 1])

            # B/C rows (later broadcast per state): [32, L]
            p_bc = pers.tile([2 * DS, L], bdt)
            # DRAM staging copy (DMA partition-broadcast needs a DRAM source)
            bc_dram = nc.dram_tensor("bc_stage", [2 * DS, L], bdt)
            # gated ssm output
            y_gated = pers.tile([DI, L], F32)

            with tc.tile_pool(name="smid", bufs=1) as smid:
                z_sil = smid.tile([DI, L], F32)
                xc = smid.tile([DI, L], F32)
                dtv = smid.tile([DI, L], F32)
                dtxc = smid.tile([DI, L], F32)
                xdt = BF16 if DBX_BF16 else F32
                dtxc_bf = (smid.tile([DI, L], xdt, name="dtxc_bf")
                           if DBX_BF16 else dtxc)

                with tc.tile_pool(name="ph12", bufs=1) as p12, \
                     tc.tile_pool(name="psA", bufs=3, space="PSUM") as psA:
                    x_sb = p12.tile([128, L], F32)
                    nc.gpsimd.memset(x_sb[64:128, :], 0.0)
                    nc.sync.dma_start(x_sb[0:64, :], x_loc[:])
                    xi_pad = p12.tile([DI, 3 + L], F32)
                    nc.gpsimd.memset(xi_pad[:, 0:3], 0.0)

                    # phase 1: xz projection + silu(z)
                    for c in range(NCH):
                        sl = slice(c * CH, (c + 1) * CH)
                        ps = psA.tile([128, CH], F32, tag="ps")
                        nc.tensor.matmul(ps[:DI], p_in_wT[:, 0:DI],
                                         x_sb[:, sl], start=True, stop=True)
                        nc.vector.tensor_copy(xi_pad[:, 3 + c * CH:3 + (c + 1) * CH],
                                              ps[:DI])
                        ps2 = psA.tile([128, CH], F32, tag="ps")
                        nc.tensor.matmul(ps2[:DI], p_in_wT[:, DI:2 * DI],
                                         x_sb[:, sl], start=True, stop=True)
                        nc.scalar.activation(z_sil[:, sl], ps2[:DI], AF.Silu)

                    # phase 2: causal depthwise conv1d + silu
                    for c in range(NCH):
                        sl = slice(c * CH, (c + 1) * CH)
                        ps = psA.tile([128, CH], F32, tag="ps")
                        for k in range(DCONV):
                            nc.tensor.matmul(
                                ps[:DI], diag_c1[k][:],
                                xi_pad[:, c * CH + k:c * CH + k + CH],
                                start=(k == 0), stop=(k == DCONV - 1))
                        nc.scalar.activation(xc[:, sl], ps[:DI], AF.Silu,
                                             bias=p_c1b[:, 0:1])

                    # phase 3: dt pre-activation + B/C projection
                    for c in range(NCH):
                        sl = slice(c * CH, (c + 1) * CH)
                        ps = psA.tile([128, CH], F32, tag="ps")
                        nc.tensor.matmul(ps[:DI], p_bigT[:], xc[:, sl],
                                         start=True, stop=True)
                        # softplus = ln(1 + exp(.))
                        nc.scalar.activation(dtv[:, sl], ps[:DI], AF.Exp,
                                             bias=p_dtb[:, 0:1])
                        nc.scalar.activation(dtv[:, sl], dtv[:, sl], AF.Ln,
                                             bias=1.0)
                        ps2 = psA.tile([128, CH], F32, tag="ps")
                        nc.tensor.matmul(ps2[:2 * DS], p_bcwT[:], xc[:, sl],
                                         start=True, stop=True)
                        nc.vector.tensor_copy(p_bc[:, sl], ps2[:2 * DS])

                    # phase 4: dt * xc
                    nc.vector.tensor_mul(dtxc[:], dtv[:], xc[:])
                    nc.sync.dma_start(bc_dram[:], p_bc[:])
                    if DBX_BF16:
                        nc.vector.tensor_copy(dtxc_bf[:], dtxc[:])

                # ---- selective scan over 16 states ----
                with tc.tile_pool(name="yps", bufs=1, space="PSUM") as yps, \
                     tc.tile_pool(name="sl_a", bufs=3) as pla, \
                     tc.tile_pool(name="sl_b", bufs=3) as plb, \
                     tc.tile_pool(name="sl_x", bufs=3) as plx, \
                     tc.tile_pool(name="sl_h", bufs=2) as plh, \
                     tc.tile_pool(name="sl_c", bufs=3) as plc, \
                     tc.tile_pool(name="sl_g", bufs=2) as plg:
                    y_ps = [yps.tile([128, CH], F32, tag=f"y{c}", name=f"y{c}")
                            for c in range(NCH)]
                    # D * xc seeds the accumulators
                    for c in range(NCH):
                        nc.tensor.matmul(y_ps[c][:DI], diag_d[:],
                                         xc[:, c * CH:(c + 1) * CH],
                                         start=True, stop=False)

                    for s in range(DS):
                        h_prev = None
                        for hf in range(NHALF):
                            it = s * NHALF + hf
                            hsl = slice(hf * HALF, (hf + 1) * HALF)
                            da = pla.tile([DI, HALF], F32, tag="da")
                            nc.scalar.activation(da[:], dtv[:, hsl], AF.Exp,
                                                 scale=p_A[:, s:s + 1])
                            bbc = plb.tile([DI, HALF], bdt, tag="bbc")
                            nc.sync.dma_start(
                                bbc[:],
                                bc_dram[s:s + 1, hsl].to_broadcast((DI, HALF)))
                            dbx = plx.tile([DI, HALF], xdt, tag="dbx")
                            nc.gpsimd.tensor_tensor(dbx[:], dtxc_bf[:, hsl],
                                                    bbc[:], op=OP.mult)
                            h = plh.tile([DI, HALF], xdt, tag="h")
                            init = 0.0 if hf == 0 else h_prev[:, HALF - 1:HALF]
                            nc.vector.tensor_tensor_scan(
                                h[:], da[:], dbx[:], init,
                                op0=OP.mult, op1=OP.add)
                            cbc = plc.tile([DI, HALF], bdt, tag="cbc")
                            nc.sync.dma_start(
                                cbc[:],
                                bc_dram[DS + s:DS + s + 1,
                                        hsl].to_broadcast((DI, HALF)))
                            g = plg.tile([DI, HALF], gdt, tag="g")
                            if (it % 8) < G_ON_GP:
                                nc.gpsimd.tensor_tensor(g[:], h[:], cbc[:],
                                                        op=OP.mult)
                            else:
                                nc.vector.tensor_mul(g[:], h[:], cbc[:])
                            for j in range(HALF // CH):
                                idx = hf * (HALF // CH) + j
                                nc.tensor.matmul(
                                    y_ps[idx][:DI], ident_g[:],
                                    g[:, j * CH:(j + 1) * CH],
                                    start=False, stop=(s == DS - 1))
                            h_prev = h

                    # gating with silu(z)
                    for c in range(NCH):
                        sl = slice(c * CH, (c + 1) * CH)
                        nc.vector.tensor_mul(y_gated[:, sl], y_ps[c][:DI],
                                             z_sil[:, sl])

            # ---- tail: out-proj, 3x3 conv partial, collectives, BN ----
            with tc.tile_pool(name="tail", bufs=1) as tl, \
                 tc.tile_pool(name="psB", bufs=3, space="PSUM") as psB, \
                 tc.tile_pool(name="dram", bufs=1, space="DRAM") as dr:
                ympad = tl.tile([128, H + 2, W + 2], c3dt)
                nc.gpsimd.memset(ympad[:], 0.0)
                for c in range(NCH):
                    ps = psB.tile([C, CH], F32, tag="psb")
                    nc.tensor.matmul(ps[:], p_owT[:], y_gated[:, c * CH:(c + 1) * CH],
                                     start=True, stop=True)
                    r0 = c * (CH // W)
                    nc.scalar.activation(
                        ympad[0:C, 1 + r0:1 + r0 + CH // W, 1:1 + W],
                        ps.rearrange("p (r w) -> p r w", w=W), AF.Copy)

                conv_part = tl.tile([C, L], F32)
                for c in range(NCH):
                    ps = psB.tile([C, CH], F32, tag="psb")
                    ps3 = ps.rearrange("p (r w) -> p r w", w=W)
                    r0 = c * (CH // W)
                    n = 0
                    for ky in range(3):
                        for kx in range(3):
                            nc.tensor.matmul(
                                ps3[:],
                                p_c3w[:, (ky * 3 + kx) * C:(ky * 3 + kx + 1) * C],
                                ympad[:, r0 + ky:r0 + ky + CH // W, kx:kx + W],
                                start=(n == 0), stop=(n == 8))
                            n += 1
                    nc.scalar.activation(conv_part[:, c * CH:(c + 1) * CH],
                                         ps3.rearrange("p r w -> p (r w)"),
                                         AF.Identity, bias=p_c3b[:, 0:1])

                # pair AllReduce of conv partials
                cc_in = dr.tile([C, L], F32)
                cc_out = dr.tile([C, L], F32)
                nc.sync.dma_start(cc_in[:], conv_part[:])
                nc.gpsimd.collective_compute(
                    "AllReduce", OP.add,
                    replica_groups=[[0, 1], [2, 3], [4, 5], [6, 7]],
                    ins=[cc_in[:].opt()], outs=[cc_out[:].opt()])
                conv_full = tl.tile([C, L], F32)
                nc.sync.dma_start(conv_full[:], cc_out[:])

                # BN statistics (local sums then 8-core AllReduce)
                stats = tl.tile([C, 2], F32)
                sq = tl.tile([C, L], F32)
                nc.vector.tensor_reduce(stats[:, 0:1], conv_full[:],
                                        axis=mybir.AxisListType.X, op=OP.add)
                nc.scalar.activation(sq[:], conv_full[:], AF.Square,
                                     accum_out=stats[:, 1:2])
                st_in = dr.tile([C, 2], F32)
                st_out = dr.tile([C, 2], F32)
                nc.sync.dma_start(st_in[:], stats[:])
                nc.gpsimd.collective_compute(
                    "AllReduce", OP.add,
                    replica_groups=[[0, 1, 2, 3, 4, 5, 6, 7]],
                    ins=[st_in[:].opt()], outs=[st_out[:].opt()])
                stot = tl.tile([C, 2], F32)
                nc.sync.dma_start(stot[:], st_out[:])

                inv_n = 1.0 / (2.0 * B * L)
                mean = tl.tile([C, 1], F32)
                var = tl.tile([C, 1], F32)
                tmp = tl.tile([C, 1], F32)
                nc.vector.tensor_scalar_mul(mean[:], stot[:, 0:1], inv_n)
                nc.vector.tensor_scalar_mul(var[:], stot[:, 1:2], inv_n)
                nc.vector.tensor_mul(tmp[:], mean[:], mean[:])
                nc.vector.tensor_sub(var[:], var[:], tmp[:])
                # invstd = 1/sqrt(var + eps)
                nc.vector.tensor_scalar_add(var[:], var[:], 1e-5)
                nc.scalar.activation(tmp[:], var[:], AF.Sqrt)
                nc.vector.reciprocal(tmp[:], tmp[:])
                scal = tl.tile([C, 1], F32)
                shft = tl.tile([C, 1], F32)
                nc.vector.tensor_mul(scal[:], p_bng[:], tmp[:])
                nc.vector.tensor_mul(tmp[:], mean[:], scal[:])
                nc.vector.tensor_sub(shft[:], p_bnb[:], tmp[:])

                # residual projection
                x2 = tl.tile([128, L], F32)
                nc.gpsimd.memset(x2[64:128, :], 0.0)
                nc.sync.dma_start(x2[0:64, :], x_loc[:])
                res_sb = tl.tile([C, L], F32)
                for c in range(NCH):
                    ps = psB.tile([C, CH], F32, tag="psb")
                    nc.tensor.matmul(ps[:], p_rwT[:], x2[:, c * CH:(c + 1) * CH],
                                     start=True, stop=True)
                    nc.scalar.activation(res_sb[:, c * CH:(c + 1) * CH], ps[:],
                                         AF.Identity, bias=p_rb[:, 0:1])

                # bn + residual + leaky relu
                bn1 = tl.tile([C, L], F32)
                nc.scalar.activation(bn1[:], conv_full[:], AF.Identity,
                                     scale=scal[:, 0:1], bias=shft[:, 0:1])
                nc.vector.tensor_add(bn1[:], bn1[:], res_sb[:])
                outt = tl.tile([C, L], F32)
                nc.scalar.activation(outt[:], bn1[:], AF.Prelu, alpha=0.01)
                nc.sync.dma_start(out_d[:], outt[:])

    nc.compile()
    return nc


_NC = None


def _get_nc():
    global _NC
    if _NC is None:
        _NC = _build()
    return _NC


def _prep_in_maps(inp):
    inp = {k: np.asarray(v, dtype=np.float32) for k, v in inp.items()}
    x = inp["x"]  # (4, 64, 64, 64)
    maps = []
    c3dtype = ml_dtypes.bfloat16 if CONV3_BF16 else np.float32
    for core in range(NCORE):
        b, d = core // 2, core % 2
        pre = "m1_" if d == 0 else "m2_"
        in_w = inp[pre + "in_w"]          # (256, 64)
        xproj_w = inp[pre + "xproj_w"]    # (36, 128)
        dt_w = inp[pre + "dt_w"]          # (128, 4)

        x_loc = x[b].reshape(C, L)
        if d == 1:
            x_loc = x_loc[:, ::-1]

        in_wT = np.zeros((128, 2 * DI), np.float32)
        in_wT[:C] = in_w.T
        bigproj = dt_w @ xproj_w[:DTR]    # (128, 128)
        conv3_slice = inp["conv_w"][:, d * C:(d + 1) * C]  # (64,64,3,3)
        c3 = np.zeros((128, 9 * C), np.float32)
        for ky in range(3):
            for kx in range(3):
                c3[:C, (ky * 3 + kx) * C:(ky * 3 + kx + 1) * C] = \
                    conv3_slice[:, :, ky, kx].T
        res_wT = np.zeros((128, C), np.float32)
        res_wT[:C] = inp["res_w"].T

        m = {
            "x_loc": np.ascontiguousarray(x_loc),
            "in_wT": in_wT,
            "conv1_w": inp[pre + "conv_w"],
            "conv1_b": inp[pre + "conv_b"].reshape(DI, 1),
            "bigproj_T": np.ascontiguousarray(bigproj.T),
            "bc_wT": np.ascontiguousarray(xproj_w[DTR:].T),
            "dt_b": inp[pre + "dt_b"].reshape(DI, 1),
            "A_mat": -np.exp(inp[pre + "A_log"]),
            "Dvec": inp[pre + "D"].reshape(DI, 1),
            "out_wT": np.ascontiguousarray(inp[pre + "out_w"].T),
            "conv3_w": c3.astype(c3dtype),
            "conv3_b": (inp["conv_b"] if d == 0
                        else np.zeros_like(inp["conv_b"])).reshape(C, 1),
            "res_wT": res_wT,
            "res_b": inp["res_b"].reshape(C, 1),
            "bn_g": inp["bn_gamma"].reshape(C, 1),
            "bn_b": inp["bn_beta"].reshape(C, 1),
        }
        maps.append(m)
    return maps


def _run(inputs, trace=False):
    nc = _get_nc()
    maps = _prep_in_maps(inputs)
    res = bass_utils.run_bass_kernel_spmd(
        nc, maps, core_ids=list(range(NCORE)), trace=trace)
    out = np.stack([res.results[2 * b]["out"].reshape(C, H, W)
                    for b in range(B)])
    return out, res


def kernel(**inputs) -> np.ndarray:
    out, _ = _run(inputs, trace=False)
    return out


# revision 22
# speedup vs baseline: 1.2276x; 1.2276x over previous
"""BiMamba block kernel for 8 Trainium2 NeuronCores.

Sharding: core = 2*sample + direction (4 samples x 2 scan directions).
Each core runs the full mamba for its (sample, direction): input/gate
projections (PE), causal depthwise conv (PE diag-matmul), dt softplus
(ACT), then 16 selective-scan states via the DVE tensor_tensor_scan
instruction, with exp(dt*A) on ACT, the dt*x*B / C*h products on DVE in
bf16, and the sum over states accumulated in PSUM by identity matmuls.
The sequence is processed in two halves so the tail of half 0 (gating,
out-projection, partial 3x3 conv) overlaps the scan of half 1.  The
block tail (3x3 conv partials, BatchNorm batch stats, residual,
LeakyReLU) uses a pair AllReduce plus an 8-core stats AllReduce.
"""
import os
import sys

for _p in ("/opt/trn_rl_repo", "/root/.axon_site/_ro/trn_rl_repo"):
    if os.path.isdir(_p):
        if _p not in sys.path:
            sys.path.insert(0, _p)
        break

import ml_dtypes
import numpy as np

# The agent image's antenv lacks axon_hooks; inject it so trace=True can
# capture NTFF profiles (used by test.py for HW timing, not for grading).
try:
    import antenv.axon_hooks  # noqa: F401
except ImportError:
    try:
        import types as _types

        from trn_agent_boot.trn_boot import _ntff_profile_via_ctypes

        _hook = _ntff_profile_via_ctypes("/opt/axon/libaxon_pjrt.so")
        _m = _types.ModuleType("antenv.axon_hooks")
        _m.get_axon_ntff_profile_hook = lambda: _hook
        _m.set_axon_ntff_profile_hook = lambda h: None
        sys.modules["antenv.axon_hooks"] = _m
    except Exception:
        pass

import concourse.bass as bass
import concourse.mybir as mybir
from concourse import bacc
from concourse import bass_utils
from concourse.masks import make_identity
from concourse.tile import TileContext

F32 = mybir.dt.float32
BF16 = mybir.dt.bfloat16
AF = mybir.ActivationFunctionType
OP = mybir.AluOpType

B, C, H, W = 4, 64, 64, 64
L = H * W          # 4096
DI = 128           # d_inner
DS = 16            # d_state
DTR = 4            # dt_rank
DCONV = 4
NCORE = 8
CH = 512           # matmul free-dim chunk
NCH = L // CH      # 8
HALF = L // 2      # scan chunk length
NHALF = 2
RPC = CH // W      # output rows per chunk (8)


def _build():
    nc = bacc.Bacc(target_bir_lowering=False, debug=False, num_devices=NCORE)

    def din(name, shape, dtype=F32):
        return nc.dram_tensor(name, shape, dtype, kind="ExternalInput")

    x_loc = din("x_loc", [C, L])
    in_wT = din("in_wT", [128, 2 * DI])       # padded K (rows 64:128 zero)
    conv1_w = din("conv1_w", [DI, DCONV])
    conv1_b = din("conv1_b", [DI, 1])
    bigproj_T = din("bigproj_T", [DI, DI])    # (dt_w @ xproj_w[:4]).T
    bc_wT = din("bc_wT", [DI, 2 * DS])        # xproj_w[4:36].T
    dt_b = din("dt_b", [DI, 1])
    A_mat = din("A_mat", [DI, DS])
    Dvec = din("Dvec", [DI, 1])
    out_wT = din("out_wT", [DI, C], BF16)
    conv3_w = din("conv3_w", [128, 9 * C], BF16)  # tap-major lhsT, padded K
    conv3_b = din("conv3_b", [C, 1])
    res_wT = din("res_wT", [128, C])          # padded K
    res_b = din("res_b", [C, 1])
    bn_g = din("bn_g", [C, 1])
    bn_b = din("bn_b", [C, 1])

    out_d = nc.dram_tensor("out", [C, L], F32, kind="ExternalOutput")

    with TileContext(nc) as tc:
        with tc.tile_pool(name="pers", bufs=1) as pers:
            # ---- small persistent tiles (params) ----
            p_in_wT = pers.tile([128, 2 * DI], F32)
            p_c1w = pers.tile([DI, DCONV], F32)
            p_c1b = pers.tile([DI, 1], F32)
            p_bigT = pers.tile([DI, DI], F32)
            p_bcwT = pers.tile([DI, 2 * DS], F32)
            p_dtb = pers.tile([DI, 1], F32)
            p_A = pers.tile([DI, DS], F32)
            p_D = pers.tile([DI, 1], F32)
            p_owT = pers.tile([DI, C], BF16)
            p_c3w = pers.tile([128, 9 * C], BF16)
            p_c3b = pers.tile([C, 1], F32)
            p_rwT = pers.tile([128, C], F32)
            p_rb = pers.tile([C, 1], F32)
            p_bng = pers.tile([C, 1], F32)
            p_bnb = pers.tile([C, 1], F32)
            for t, d in ((p_in_wT, in_wT), (p_c1w, conv1_w), (p_c1b, conv1_b),
                         (p_bigT, bigproj_T), (p_bcwT, bc_wT), (p_dtb, dt_b),
                         (p_A, A_mat), (p_D, Dvec), (p_owT, out_wT),
                         (p_c3w, conv3_w), (p_c3b, conv3_b), (p_rwT, res_wT),
                         (p_rb, res_b), (p_bng, bn_g), (p_bnb, bn_b)):
                nc.sync.dma_start(t[:], d[:])

            ident = pers.tile([128, 128], F32)
            make_identity(nc, ident[:])
            ident_g = pers.tile([128, 128], BF16)
            nc.vector.tensor_copy(ident_g[:], ident[:])
            diag_c1 = [pers.tile([128, 128], F32, tag=f"dgc{k}", name=f"dgc{k}")
                       for k in range(DCONV)]
            for k in range(DCONV):
                nc.vector.tensor_scalar_mul(diag_c1[k][:], ident[:],
                                            p_c1w[:, k:k + 1])

            # B/C rows (later broadcast per state): [32, L]
            p_bc = pers.tile([2 * DS, L], BF16)
            # DRAM staging copy (DMA partition-broadcast needs a DRAM source)
            bc_dram = nc.dram_tensor("bc_stage", [2 * DS, L], BF16)
            y_gated = pers.tile([DI, L], BF16)
            res_sb = pers.tile([C, L], F32)

            with tc.tile_pool(name="smid", bufs=1) as smid:
                z_sil = smid.tile([DI, L], BF16)
                dtv = smid.tile([DI, L], F32)
                dtxc_bf = smid.tile([DI, L], BF16)
                xcd = smid.tile([DI, L], BF16)
                carry = smid.tile([DI, DS], F32)

                with tc.tile_pool(name="ph12", bufs=1) as p12, \
                     tc.tile_pool(name="psA", bufs=3, space="PSUM") as psA:
                    x_sb = p12.tile([128, L], F32)
                    nc.gpsimd.memset(x_sb[64:128, :], 0.0)
                    nc.sync.dma_start(x_sb[0:64, :], x_loc[:])
                    xi_pad = p12.tile([DI, 3 + L], F32)
                    nc.gpsimd.memset(xi_pad[:, 0:3], 0.0)
                    xc = p12.tile([DI, L], F32)

                    # phase 1: xz projection + silu(z); residual projection
                    for c in range(NCH):
                        sl = slice(c * CH, (c + 1) * CH)
                        ps = psA.tile([128, CH], F32, tag="ps")
                        nc.tensor.matmul(ps[:DI], p_in_wT[:, 0:DI],
                                         x_sb[:, sl], start=True, stop=True)
                        nc.scalar.copy(xi_pad[:, 3 + c * CH:3 + (c + 1) * CH],
                                       ps[:DI])
                        ps2 = psA.tile([128, CH], F32, tag="ps")
                        nc.tensor.matmul(ps2[:DI], p_in_wT[:, DI:2 * DI],
                                         x_sb[:, sl], start=True, stop=True)
                        nc.scalar.activation(z_sil[:, sl], ps2[:DI], AF.Silu)
                        ps3 = psA.tile([128, CH], F32, tag="ps")
                        nc.tensor.matmul(ps3[:C], p_rwT[:], x_sb[:, sl],
                                         start=True, stop=True)
                        nc.scalar.activation(res_sb[:, sl], ps3[:C],
                                             AF.Identity, bias=p_rb[:, 0:1])

                    # phase 2: causal depthwise conv1d + silu
                    for c in range(NCH):
                        sl = slice(c * CH, (c + 1) * CH)
                        ps = psA.tile([128, CH], F32, tag="ps")
                        for k in range(DCONV):
                            nc.tensor.matmul(
                                ps[:DI], diag_c1[k][:],
                                xi_pad[:, c * CH + k:c * CH + k + CH],
                                start=(k == 0), stop=(k == DCONV - 1))
                        nc.scalar.activation(xc[:, sl], ps[:DI], AF.Silu,
                                             bias=p_c1b[:, 0:1])

                    # phase 3: dt pre-activation + B/C projection
                    for c in range(NCH):
                        sl = slice(c * CH, (c + 1) * CH)
                        ps = psA.tile([128, CH], F32, tag="ps")
                        nc.tensor.matmul(ps[:DI], p_bigT[:], xc[:, sl],
                                         start=True, stop=True)
                        # softplus = ln(1 + exp(.))
                        nc.scalar.activation(dtv[:, sl], ps[:DI], AF.Exp,
                                             bias=p_dtb[:, 0:1])
                        nc.scalar.activation(dtv[:, sl], dtv[:, sl], AF.Ln,
                                             bias=1.0)
                        ps2 = psA.tile([128, CH], F32, tag="ps")
                        nc.tensor.matmul(ps2[:2 * DS], p_bcwT[:], xc[:, sl],
                                         start=True, stop=True)
                        nc.scalar.copy(p_bc[:, sl], ps2[:2 * DS])

                    # phase 4: dt*xc (bf16) and xc*D (bf16)
                    nc.vector.tensor_mul(dtxc_bf[:], dtv[:], xc[:])
                    nc.scalar.activation(xcd[:], xc[:], AF.Copy,
                                         scale=p_D[:, 0:1])
                    nc.sync.dma_start(bc_dram[:], p_bc[:])

                # ---- selective scan + overlapped tail ----
                with tc.tile_pool(name="pp", bufs=8, space="PSUM") as pp, \
                     tc.tile_pool(name="sl_a", bufs=2) as pla, \
                     tc.tile_pool(name="sl_b", bufs=2) as plb, \
                     tc.tile_pool(name="sl_x", bufs=2) as plx, \
                     tc.tile_pool(name="sl_h", bufs=2) as plh, \
                     tc.tile_pool(name="sl_c", bufs=2) as plc, \
                     tc.tile_pool(name="sl_g", bufs=2) as plg, \
                     tc.tile_pool(name="tail", bufs=1) as tl, \
                     tc.tile_pool(name="tbig", bufs=1) as tbig, \
                     tc.tile_pool(name="dram", bufs=1, space="DRAM") as dr:
                    ympad = tl.tile([128, H + 2, W + 2], BF16)
                    nc.gpsimd.memset(ympad[:], 0.0)
                    conv_part = tl.tile([C, L], F32)

                    def conv3_chunk(c):
                        ps = pp.tile([128, CH], F32, tag="pp",
                                     name=f"cv{c}")
                        ps3 = ps[:C].rearrange("p (r w) -> p r w", w=W)
                        r0 = c * RPC
                        n = 0
                        for ky in range(3):
                            for kx in range(3):
                                nc.tensor.matmul(
                                    ps3[:],
                                    p_c3w[:, (ky * 3 + kx) * C:
                                          (ky * 3 + kx + 1) * C],
                                    ympad[:, r0 + ky:r0 + ky + RPC, kx:kx + W],
                                    start=(n == 0), stop=(n == 8))
                                n += 1
                        nc.scalar.activation(conv_part[:, c * CH:(c + 1) * CH],
                                             ps3.rearrange("p r w -> p (r w)"),
                                             AF.Identity, bias=p_c3b[:, 0:1])

                    for hf in range(NHALF):
                        y_ps = []
                        for j in range(NCH // 2):
                            cix = hf * (NCH // 2) + j
                            yp = pp.tile([128, CH], F32, tag="pp",
                                         name=f"y{cix}")
                            nc.tensor.matmul(
                                yp[:DI], ident_g[:],
                                xcd[:, cix * CH:(cix + 1) * CH],
                                start=True, stop=False)
                            y_ps.append(yp)

                        hsl = slice(hf * HALF, (hf + 1) * HALF)
                        for s in range(DS):
                            da = pla.tile([DI, HALF], F32, tag="da")
                            nc.scalar.activation(da[:], dtv[:, hsl], AF.Exp,
                                                 scale=p_A[:, s:s + 1])
                            bbc = plb.tile([DI, HALF], BF16, tag="bbc")
                            nc.sync.dma_start(
                                bbc[:],
                                bc_dram[s:s + 1, hsl].to_broadcast((DI, HALF)))
                            dbx = plx.tile([DI, HALF], BF16, tag="dbx")
                            nc.vector.tensor_mul(dbx[:], dtxc_bf[:, hsl],
                                                 bbc[:])
                            h = plh.tile([DI, HALF], BF16, tag="h")
                            init = 0.0 if hf == 0 else carry[:, s:s + 1]
                            nc.vector.tensor_tensor_scan(
                                h[:], da[:], dbx[:], init,
                                op0=OP.mult, op1=OP.add)
                            if hf == 0:
                                nc.vector.tensor_copy(carry[:, s:s + 1],
                                                      h[:, HALF - 1:HALF])
                            cbc = plc.tile([DI, HALF], BF16, tag="cbc")
                            nc.sync.dma_start(
                                cbc[:],
                                bc_dram[DS + s:DS + s + 1,
                                        hsl].to_broadcast((DI, HALF)))
                            g = plg.tile([DI, HALF], BF16, tag="g")
                            nc.vector.tensor_mul(g[:], h[:], cbc[:])
                            for j in range(NCH // 2):
                                nc.tensor.matmul(
                                    y_ps[j][:DI], ident_g[:],
                                    g[:, j * CH:(j + 1) * CH],
                                    start=False, stop=(s == DS - 1))

                        # gating + out-projection + padded spatial write
                        for j in range(NCH // 2):
                            cix = hf * (NCH // 2) + j
                            sl = slice(cix * CH, (cix + 1) * CH)
                            nc.vector.tensor_mul(y_gated[:, sl],
                                                 y_ps[j][:DI], z_sil[:, sl])
                            po = pp.tile([128, CH], F32, tag="pp",
                                         name=f"po{cix}")
                            nc.tensor.matmul(po[:C], p_owT[:],
                                             y_gated[:, sl],
                                             start=True, stop=True)
                            r0 = cix * RPC
                            nc.scalar.copy(
                                ympad[0:C, 1 + r0:1 + r0 + RPC, 1:1 + W],
                                po[:C].rearrange("p (r w) -> p r w", w=W))

                        # 3x3 conv on rows whose inputs are complete
                        if hf == 0:
                            for c in range(0, 3):
                                conv3_chunk(c)
                        else:
                            for c in range(3, NCH):
                                conv3_chunk(c)

                    # pair AllReduce of conv partials
                    cc_in = dr.tile([C, L], F32)
                    cc_out = dr.tile([C, L], F32)
                    nc.sync.dma_start(cc_in[:], conv_part[:])
                    nc.gpsimd.collective_compute(
                        "AllReduce", OP.add,
                        replica_groups=[[0, 1], [2, 3], [4, 5], [6, 7]],
                        ins=[cc_in[:].opt()], outs=[cc_out[:].opt()])
                    conv_full = tl.tile([C, L], F32)
                    nc.sync.dma_start(conv_full[:], cc_out[:])

                    # BN statistics (local sums then 8-core AllReduce)
                    stats = tl.tile([C, 2], F32)
                    sq = tl.tile([C, L], F32, name="sq")
                    nc.vector.tensor_reduce(stats[:, 0:1], conv_full[:],
                                            axis=mybir.AxisListType.X,
                                            op=OP.add)
                    nc.scalar.activation(sq[:], conv_full[:], AF.Square,
                                         accum_out=stats[:, 1:2])
                    st_in = dr.tile([C, 2], F32)
                    st_out = dr.tile([C, 2], F32)
                    nc.sync.dma_start(st_in[:], stats[:])
                    nc.gpsimd.collective_compute(
                        "AllReduce", OP.add,
                        replica_groups=[[0, 1, 2, 3, 4, 5, 6, 7]],
                        ins=[st_in[:].opt()], outs=[st_out[:].opt()])
                    stot = tl.tile([C, 2], F32)
                    nc.sync.dma_start(stot[:], st_out[:])

                    inv_n = 1.0 / (2.0 * B * L)
                    mean = tl.tile([C, 1], F32)
                    var = tl.tile([C, 1], F32)
                    tmp = tl.tile([C, 1], F32)
                    nc.vector.tensor_scalar_mul(mean[:], stot[:, 0:1], inv_n)
                    nc.vector.tensor_scalar_mul(var[:], stot[:, 1:2], inv_n)
                    nc.vector.tensor_mul(tmp[:], mean[:], mean[:])
                    nc.vector.tensor_sub(var[:], var[:], tmp[:])
                    # invstd = 1/sqrt(var + eps)
                    nc.vector.tensor_scalar_add(var[:], var[:], 1e-5)
                    nc.scalar.activation(tmp[:], var[:], AF.Sqrt)
                    nc.vector.reciprocal(tmp[:], tmp[:])
                    scal = tl.tile([C, 1], F32)
                    shft = tl.tile([C, 1], F32)
                    nc.vector.tensor_mul(scal[:], p_bng[:], tmp[:])
                    nc.vector.tensor_mul(tmp[:], mean[:], scal[:])
                    nc.vector.tensor_sub(shft[:], p_bnb[:], tmp[:])

                    # bn + residual + leaky relu
                    bn1 = tbig.tile([C, L], F32, tag="big", name="bn1")
                    nc.scalar.activation(bn1[:], conv_full[:], AF.Identity,
                                         scale=scal[:, 0:1], bias=shft[:, 0:1])
                    nc.vector.tensor_add(bn1[:], bn1[:], res_sb[:])
                    nc.scalar.activation(bn1[:], bn1[:], AF.Prelu, alpha=0.01)
                    nc.sync.dma_start(out_d[:], bn1[:])

    nc.compile()
    return nc


_NC = None


def _get_nc():
    global _NC
    if _NC is None:
        _NC = _build()
    return _NC


def _prep_in_maps(inp):
    inp = {k: np.asarray(v, dtype=np.float32) for k, v in inp.items()}
    x = inp["x"]  # (4, 64, 64, 64)
    maps = []
    for core in range(NCORE):
        b, d = core // 2, core % 2
        pre = "m1_" if d == 0 else "m2_"
        in_w = inp[pre + "in_w"]          # (256, 64)
        xproj_w = inp[pre + "xproj_w"]    # (36, 128)
        dt_w = inp[pre + "dt_w"]          # (128, 4)

        x_loc = x[b].reshape(C, L)
        if d == 1:
            x_loc = x_loc[:, ::-1]

        in_wT = np.zeros((128, 2 * DI), np.float32)
        in_wT[:C] = in_w.T
        bigproj = dt_w @ xproj_w[:DTR]    # (128, 128)
        conv3_slice = inp["conv_w"][:, d * C:(d + 1) * C]  # (64,64,3,3)
        c3 = np.zeros((128, 9 * C), np.float32)
        for ky in range(3):
            for kx in range(3):
                c3[:C, (ky * 3 + kx) * C:(ky * 3 + kx + 1) * C] = \
                    conv3_slice[:, :, ky, kx].T
        res_wT = np.zeros((128, C), np.float32)
        res_wT[:C] = inp["res_w"].T

        m = {
            "x_loc": np.ascontiguousarray(x_loc),
            "in_wT": in_wT,
            "conv1_w": inp[pre + "conv_w"],
            "conv1_b": inp[pre + "conv_b"].reshape(DI, 1),
            "bigproj_T": np.ascontiguousarray(bigproj.T),
            "bc_wT": np.ascontiguousarray(xproj_w[DTR:].T),
            "dt_b": inp[pre + "dt_b"].reshape(DI, 1),
            "A_mat": -np.exp(inp[pre + "A_log"]),
            "Dvec": inp[pre + "D"].reshape(DI, 1),
            "out_wT": inp[pre + "out_w"].T.astype(ml_dtypes.bfloat16),
            "conv3_w": c3.astype(ml_dtypes.bfloat16),
            "conv3_b": (inp["conv_b"] if d == 0
                        else np.zeros_like(inp["conv_b"])).reshape(C, 1),
            "res_wT": res_wT,
            "res_b": inp["res_b"].reshape(C, 1),
            "bn_g": inp["bn_gamma"].reshape(C, 1),
            "bn_b": inp["bn_beta"].reshape(C, 1),
        }
        maps.append(m)
    return maps


def _run(inputs, trace=False):
    nc = _get_nc()
    maps = _prep_in_maps(inputs)
    res = bass_utils.run_bass_kernel_spmd(
        nc, maps, core_ids=list(range(NCORE)), trace=trace)
    out = np.stack([res.results[2 * b]["out"].reshape(C, H, W)
                    for b in range(B)])
    return out, res


def kernel(**inputs) -> np.ndarray:
    out, _ = _run(inputs, trace=False)
    return out


# revision 27
# speedup vs baseline: 1.4253x; 1.1610x over previous
"""BiMamba block kernel for 8 Trainium2 NeuronCores.

Sharding: core = 2*sample + direction (4 samples x 2 scan directions).
Each core runs the full mamba for its (sample, direction): input/gate
projections (PE), causal depthwise conv (PE diag-matmul), dt softplus
(ACT), then 16 selective-scan states via the DVE tensor_tensor_scan
instruction, with exp(dt*A) on ACT, the dt*x*B / C*h products on DVE in
bf16, and the sum over states accumulated in PSUM by identity matmuls.
The sequence is processed in two halves so the tail of half 0 (gating,
out-projection, partial 3x3 conv) overlaps the scan of half 1.  The
block tail (3x3 conv partials, BatchNorm batch stats, residual,
LeakyReLU) uses a pair AllReduce plus an 8-core stats AllReduce.
"""
import os
import sys

for _p in ("/opt/trn_rl_repo", "/root/.axon_site/_ro/trn_rl_repo"):
    if os.path.isdir(_p):
        if _p not in sys.path:
            sys.path.insert(0, _p)
        break

import ml_dtypes
import numpy as np

# The agent image's antenv lacks axon_hooks; inject it so trace=True can
# capture NTFF profiles (used by test.py for HW timing, not for grading).
try:
    import antenv.axon_hooks  # noqa: F401
except ImportError:
    try:
        import types as _types

        from trn_agent_boot.trn_boot import _ntff_profile_via_ctypes

        _hook = _ntff_profile_via_ctypes("/opt/axon/libaxon_pjrt.so")
        _m = _types.ModuleType("antenv.axon_hooks")
        _m.get_axon_ntff_profile_hook = lambda: _hook
        _m.set_axon_ntff_profile_hook = lambda h: None
        sys.modules["antenv.axon_hooks"] = _m
    except Exception:
        pass

import concourse.bass as bass
import concourse.mybir as mybir
from concourse import bacc
from concourse import bass_utils
from concourse.masks import make_identity
from concourse.tile import TileContext

F32 = mybir.dt.float32
BF16 = mybir.dt.bfloat16
AF = mybir.ActivationFunctionType
OP = mybir.AluOpType

B, C, H, W = 4, 64, 64, 64
L = H * W          # 4096
DI = 128           # d_inner
DS = 16            # d_state
DTR = 4            # dt_rank
DCONV = 4
NCORE = 8
CH = 512           # matmul free-dim chunk
NCH = L // CH      # 8
HALF = L // 2      # scan chunk length
NHALF = 2
RPC = CH // W      # output rows per chunk (8)


def _build():
    nc = bacc.Bacc(target_bir_lowering=False, debug=False, num_devices=NCORE)

    def din(name, shape, dtype=F32):
        return nc.dram_tensor(name, shape, dtype, kind="ExternalInput")

    F32R = mybir.dt.float32r
    x_loc = din("x_loc", [C, L], F32R)
    in_wT = din("in_wT", [128, 2 * DI], F32R)  # padded K (rows 64:128 zero)
    conv1_w = din("conv1_w", [DI, DCONV])
    conv1_b = din("conv1_b", [DI, 1])
    bigproj_T = din("bigproj_T", [DI, DI], F32R)  # (dt_w @ xproj_w[:4]).T
    bc_wT = din("bc_wT", [DI, 2 * DS], F32R)  # xproj_w[4:36].T
    dt_b = din("dt_b", [DI, 1])
    A_mat = din("A_mat", [DI, DS])
    Dvec = din("Dvec", [DI, 1])
    out_wT = din("out_wT", [DI, C], BF16)
    conv3_w = din("conv3_w", [128, 9 * C], BF16)  # tap-major lhsT, padded K
    conv3_b = din("conv3_b", [C, 1])
    res_wT = din("res_wT", [128, C], F32R)    # padded K
    res_b = din("res_b", [C, 1])
    bn_g = din("bn_g", [C, 1])
    bn_b = din("bn_b", [C, 1])

    out_d = nc.dram_tensor("out", [C, L], F32, kind="ExternalOutput")

    with TileContext(nc) as tc:
        with tc.tile_pool(name="pers", bufs=1) as pers:
            # ---- small persistent tiles (params) ----
            p_in_wT = pers.tile([128, 2 * DI], F32R)
            p_c1w = pers.tile([DI, DCONV], F32)
            p_c1b = pers.tile([DI, 1], F32)
            p_bigT = pers.tile([DI, DI], F32R)
            p_bcwT = pers.tile([DI, 2 * DS], F32R)
            p_dtb = pers.tile([DI, 1], F32)
            p_A = pers.tile([DI, DS], F32)
            p_D = pers.tile([DI, 1], F32)
            p_owT = pers.tile([DI, C], BF16)
            p_c3w = pers.tile([128, 9 * C], BF16)
            p_c3b = pers.tile([C, 1], F32)
            p_rwT = pers.tile([128, C], F32R)
            p_rb = pers.tile([C, 1], F32)
            p_bng = pers.tile([C, 1], F32)
            p_bnb = pers.tile([C, 1], F32)
            for t, d in ((p_in_wT, in_wT), (p_c1w, conv1_w), (p_c1b, conv1_b),
                         (p_bigT, bigproj_T), (p_bcwT, bc_wT), (p_dtb, dt_b),
                         (p_A, A_mat), (p_D, Dvec), (p_owT, out_wT),
                         (p_c3w, conv3_w), (p_c3b, conv3_b), (p_rwT, res_wT),
                         (p_rb, res_b), (p_bng, bn_g), (p_bnb, bn_b)):
                nc.sync.dma_start(t[:], d[:])

            ident = pers.tile([128, 128], F32)
            make_identity(nc, ident[:])
            ident_g = pers.tile([128, 128], BF16)
            nc.vector.tensor_copy(ident_g[:], ident[:])
            diag_c1 = [pers.tile([128, 128], F32R, tag=f"dgc{k}", name=f"dgc{k}")
                       for k in range(DCONV)]
            for k in range(DCONV):
                nc.vector.tensor_scalar_mul(diag_c1[k][:], ident[:],
                                            p_c1w[:, k:k + 1])

            # B/C rows (later broadcast per state): [32, L]
            p_bc = pers.tile([2 * DS, L], BF16)
            # DRAM staging copy (DMA partition-broadcast needs a DRAM source)
            bc_dram = nc.dram_tensor("bc_stage", [2 * DS, L], BF16)
            y_gated = pers.tile([DI, L], BF16)
            res_sb = pers.tile([C, L], F32)

            with tc.tile_pool(name="smid", bufs=1) as smid:
                z_sil = smid.tile([DI, L], BF16)
                dtv = smid.tile([DI, L], F32)
                dtxc_bf = smid.tile([DI, L], BF16)
                xcd = smid.tile([DI, L], BF16)
                carry = smid.tile([DI, DS], F32)

                with tc.tile_pool(name="ph12", bufs=1) as p12, \
                     tc.tile_pool(name="psA", bufs=3, space="PSUM") as psA:
                    x_sb = p12.tile([128, L], F32R)
                    nc.gpsimd.memset(x_sb[64:128, :].bitcast(F32), 0.0)
                    nc.sync.dma_start(x_sb[0:64, :], x_loc[:])
                    xi_pad = p12.tile([DI, 3 + L], F32R)
                    nc.gpsimd.memset(xi_pad[:, 0:3].bitcast(F32), 0.0)
                    xc = p12.tile([DI, L], F32R)

                    # phase 1: xz projection + silu(z); residual projection
                    for c in range(NCH):
                        sl = slice(c * CH, (c + 1) * CH)
                        ps = psA.tile([128, CH], F32, tag="ps")
                        nc.tensor.matmul(ps[:DI], p_in_wT[:, 0:DI],
                                         x_sb[:, sl], start=True, stop=True)
                        nc.scalar.copy(xi_pad[:, 3 + c * CH:3 + (c + 1) * CH],
                                       ps[:DI])
                        ps2 = psA.tile([128, CH], F32, tag="ps")
                        nc.tensor.matmul(ps2[:DI], p_in_wT[:, DI:2 * DI],
                                         x_sb[:, sl], start=True, stop=True)
                        nc.scalar.activation(z_sil[:, sl], ps2[:DI], AF.Silu)
                        ps3 = psA.tile([128, CH], F32, tag="ps")
                        nc.tensor.matmul(ps3[:C], p_rwT[:], x_sb[:, sl],
                                         start=True, stop=True)
                        nc.scalar.activation(res_sb[:, sl], ps3[:C],
                                             AF.Identity, bias=p_rb[:, 0:1])

                    # phase 2: causal depthwise conv1d + silu
                    for c in range(NCH):
                        sl = slice(c * CH, (c + 1) * CH)
                        ps = psA.tile([128, CH], F32, tag="ps")
                        for k in range(DCONV):
                            nc.tensor.matmul(
                                ps[:DI], diag_c1[k][:],
                                xi_pad[:, c * CH + k:c * CH + k + CH],
                                start=(k == 0), stop=(k == DCONV - 1))
                        nc.scalar.activation(xc[:, sl], ps[:DI], AF.Silu,
                                             bias=p_c1b[:, 0:1])

                    # phase 3: dt pre-activation + B/C projection
                    # (exp batch, then ln batch, to avoid ACT table reloads)
                    xc_f = xc[:].bitcast(F32)
                    for c in range(NCH):
                        sl = slice(c * CH, (c + 1) * CH)
                        ps = psA.tile([128, CH], F32, tag="ps")
                        nc.tensor.matmul(ps[:DI], p_bigT[:], xc[:, sl],
                                         start=True, stop=True)
                        # softplus = ln(1 + exp(.))
                        nc.scalar.activation(dtv[:, sl], ps[:DI], AF.Exp,
                                             bias=p_dtb[:, 0:1])
                        ps2 = psA.tile([128, CH], F32, tag="ps")
                        nc.tensor.matmul(ps2[:2 * DS], p_bcwT[:], xc[:, sl],
                                         start=True, stop=True)
                        nc.scalar.copy(p_bc[:, sl], ps2[:2 * DS])
                    for hf2 in range(NHALF):
                        hsl2 = slice(hf2 * HALF, (hf2 + 1) * HALF)
                        nc.scalar.activation(dtv[:, hsl2], dtv[:, hsl2],
                                             AF.Ln, bias=1.0)
                        # phase 4 per half: dt*xc (bf16) and xc*D (bf16)
                        nc.vector.tensor_mul(dtxc_bf[:, hsl2], dtv[:, hsl2],
                                             xc_f[:, hsl2])
                        nc.scalar.activation(xcd[:, hsl2], xc_f[:, hsl2],
                                             AF.Copy, scale=p_D[:, 0:1])
                        nc.sync.dma_start(bc_dram[:, hsl2], p_bc[:, hsl2])

                # ---- selective scan + overlapped tail ----
                with tc.tile_pool(name="pp", bufs=8, space="PSUM") as pp, \
                     tc.tile_pool(name="sl_a", bufs=3) as pla, \
                     tc.tile_pool(name="sl_b", bufs=2) as plb, \
                     tc.tile_pool(name="sl_x", bufs=2) as plx, \
                     tc.tile_pool(name="sl_h", bufs=2) as plh, \
                     tc.tile_pool(name="sl_c", bufs=2) as plc, \
                     tc.tile_pool(name="sl_g", bufs=2) as plg, \
                     tc.tile_pool(name="tail", bufs=1) as tl, \
                     tc.tile_pool(name="tbig", bufs=1) as tbig, \
                     tc.tile_pool(name="dram", bufs=1, space="DRAM") as dr:
                    ympad = tl.tile([128, H + 2, W + 2], BF16)
                    nc.gpsimd.memset(ympad[:], 0.0)
                    conv_part = tl.tile([C, L], F32)
                    PAIRS = [[0, 1], [2, 3], [4, 5], [6, 7]]
                    cc_inA = dr.tile([C, 3 * CH], F32)
                    cc_inB = dr.tile([C, L - 3 * CH], F32)
                    cc_outA = dr.tile([C, 3 * CH], F32)
                    cc_outB = dr.tile([C, L - 3 * CH], F32)

                    def conv3_chunk(c):
                        ps = pp.tile([128, CH], F32, tag="pp",
                                     name=f"cv{c}")
                        ps3 = ps[:C].rearrange("p (r w) -> p r w", w=W)
                        r0 = c * RPC
                        n = 0
                        for ky in range(3):
                            for kx in range(3):
                                nc.tensor.matmul(
                                    ps3[:],
                                    p_c3w[:, (ky * 3 + kx) * C:
                                          (ky * 3 + kx + 1) * C],
                                    ympad[:, r0 + ky:r0 + ky + RPC, kx:kx + W],
                                    start=(n == 0), stop=(n == 8))
                                n += 1
                        nc.scalar.activation(conv_part[:, c * CH:(c + 1) * CH],
                                             ps3.rearrange("p r w -> p (r w)"),
                                             AF.Identity, bias=p_c3b[:, 0:1])

                    for hf in range(NHALF):
                        y_ps = []
                        for j in range(NCH // 2):
                            cix = hf * (NCH // 2) + j
                            yp = pp.tile([128, CH], F32, tag="pp",
                                         name=f"y{cix}")
                            nc.tensor.matmul(
                                yp[:DI], ident_g[:],
                                xcd[:, cix * CH:(cix + 1) * CH],
                                start=True, stop=False)
                            y_ps.append(yp)

                        hsl = slice(hf * HALF, (hf + 1) * HALF)
                        for s in range(DS):
                            da = pla.tile([DI, HALF], F32, tag="da")
                            nc.scalar.activation(da[:], dtv[:, hsl], AF.Exp,
                                                 scale=p_A[:, s:s + 1])
                            bbc = plb.tile([DI, HALF], BF16, tag="bbc")
                            nc.sync.dma_start(
                                bbc[:],
                                bc_dram[s:s + 1, hsl].to_broadcast((DI, HALF)))
                            dbx = plx.tile([DI, HALF], BF16, tag="dbx")
                            nc.vector.tensor_mul(dbx[:], dtxc_bf[:, hsl],
                                                 bbc[:])
                            h = plh.tile([DI, HALF], BF16, tag="h")
                            init = 0.0 if hf == 0 else carry[:, s:s + 1]
                            nc.vector.tensor_tensor_scan(
                                h[:], da[:], dbx[:], init,
                                op0=OP.mult, op1=OP.add)
                            if hf == 0:
                                nc.vector.tensor_copy(carry[:, s:s + 1],
                                                      h[:, HALF - 1:HALF])
                            cbc = plc.tile([DI, HALF], BF16, tag="cbc")
                            nc.sync.dma_start(
                                cbc[:],
                                bc_dram[DS + s:DS + s + 1,
                                        hsl].to_broadcast((DI, HALF)))
                            g = plg.tile([DI, HALF], BF16, tag="g")
                            nc.vector.tensor_mul(g[:], h[:], cbc[:])
                            for j in range(NCH // 2):
                                nc.tensor.matmul(
                                    y_ps[j][:DI], ident_g[:],
                                    g[:, j * CH:(j + 1) * CH],
                                    start=False, stop=(s == DS - 1))

                        # gating + out-projection + padded spatial write
                        for j in range(NCH // 2):
                            cix = hf * (NCH // 2) + j
                            sl = slice(cix * CH, (cix + 1) * CH)
                            nc.vector.tensor_mul(y_gated[:, sl],
                                                 y_ps[j][:DI], z_sil[:, sl])
                            po = pp.tile([128, CH], F32, tag="pp",
                                         name=f"po{cix}")
                            nc.tensor.matmul(po[:C], p_owT[:],
                                             y_gated[:, sl],
                                             start=True, stop=True)
                            r0 = cix * RPC
                            nc.scalar.copy(
                                ympad[0:C, 1 + r0:1 + r0 + RPC, 1:1 + W],
                                po[:C].rearrange("p (r w) -> p r w", w=W))

                        # 3x3 conv on rows whose inputs are complete,
                        # then kick off the pair AllReduce of that span
                        if hf == 0:
                            for c in range(0, 3):
                                conv3_chunk(c)
                            nc.sync.dma_start(cc_inA[:],
                                              conv_part[:, 0:3 * CH])
                            nc.gpsimd.collective_compute(
                                "AllReduce", OP.add,
                                replica_groups=PAIRS,
                                ins=[cc_inA[:].opt()],
                                outs=[cc_outA[:].opt()])
                        else:
                            for c in range(3, NCH):
                                conv3_chunk(c)
                            nc.sync.dma_start(cc_inB[:],
                                              conv_part[:, 3 * CH:L])
                            nc.gpsimd.collective_compute(
                                "AllReduce", OP.add,
                                replica_groups=PAIRS,
                                ins=[cc_inB[:].opt()],
                                outs=[cc_outB[:].opt()])

                    conv_full = tl.tile([C, L], F32)
                    nc.sync.dma_start(conv_full[:, 0:3 * CH], cc_outA[:])
                    nc.sync.dma_start(conv_full[:, 3 * CH:L], cc_outB[:])

                    # BN statistics (local sums then 8-core AllReduce)
                    stats = tl.tile([C, 2], F32)
                    nc.vector.tensor_reduce(stats[:, 0:1], conv_full[:],
                                            axis=mybir.AxisListType.X,
                                            op=OP.add)
                    # square scratch overwrites conv_part (dead after the
                    # collective input DMAs); only accum_out is consumed
                    nc.scalar.activation(conv_part[:], conv_full[:], AF.Square,
                                         accum_out=stats[:, 1:2])
                    st_in = dr.tile([C, 2], F32)
                    st_out = nc.dram_tensor("st_out", [C, 2], F32,
                                            addr_space="Shared")
                    nc.sync.dma_start(st_in[:], stats[:])
                    nc.gpsimd.collective_compute(
                        "AllReduce", OP.add,
                        replica_groups=[[0, 1, 2, 3, 4, 5, 6, 7]],
                        ins=[st_in[:].opt()], outs=[st_out[:].opt()])
                    stot = tl.tile([C, 2], F32)
                    nc.sync.dma_start(stot[:], st_out[:])

                    inv_n = 1.0 / (2.0 * B * L)
                    mean = tl.tile([C, 1], F32)
                    var = tl.tile([C, 1], F32)
                    tmp = tl.tile([C, 1], F32)
                    nc.vector.tensor_scalar_mul(mean[:], stot[:, 0:1], inv_n)
                    nc.vector.tensor_scalar_mul(var[:], stot[:, 1:2], inv_n)
                    nc.vector.tensor_mul(tmp[:], mean[:], mean[:])
                    nc.vector.tensor_sub(var[:], var[:], tmp[:])
                    # invstd = 1/sqrt(var + eps)
                    nc.vector.tensor_scalar_add(var[:], var[:], 1e-5)
                    nc.scalar.activation(tmp[:], var[:], AF.Sqrt)
                    nc.vector.reciprocal(tmp[:], tmp[:])
                    scal = tl.tile([C, 1], F32)
                    shft = tl.tile([C, 1], F32)
                    nc.vector.tensor_mul(scal[:], p_bng[:], tmp[:])
                    nc.vector.tensor_mul(tmp[:], mean[:], scal[:])
                    nc.vector.tensor_sub(shft[:], p_bnb[:], tmp[:])

                    # bn + residual + leaky relu
                    bn1 = tbig.tile([C, L], F32, tag="big", name="bn1")
                    nc.scalar.activation(bn1[:], conv_full[:], AF.Identity,
                                         scale=scal[:, 0:1], bias=shft[:, 0:1])
                    nc.vector.tensor_add(bn1[:], bn1[:], res_sb[:])
                    nc.scalar.activation(bn1[:], bn1[:], AF.Prelu, alpha=0.01)
                    nc.sync.dma_start(out_d[:], bn1[:])

    nc.compile()
    return nc


_NC = None


def _get_nc():
    global _NC
    if _NC is None:
        _NC = _build()
    return _NC


def _prep_in_maps(inp):
    inp = {k: np.asarray(v, dtype=np.float32) for k, v in inp.items()}
    x = inp["x"]  # (4, 64, 64, 64)
    maps = []
    for core in range(NCORE):
        b, d = core // 2, core % 2
        pre = "m1_" if d == 0 else "m2_"
        in_w = inp[pre + "in_w"]          # (256, 64)
        xproj_w = inp[pre + "xproj_w"]    # (36, 128)
        dt_w = inp[pre + "dt_w"]          # (128, 4)

        x_loc = x[b].reshape(C, L)
        if d == 1:
            x_loc = x_loc[:, ::-1]

        in_wT = np.zeros((128, 2 * DI), np.float32)
        in_wT[:C] = in_w.T
        bigproj = dt_w @ xproj_w[:DTR]    # (128, 128)
        conv3_slice = inp["conv_w"][:, d * C:(d + 1) * C]  # (64,64,3,3)
        c3 = np.zeros((128, 9 * C), np.float32)
        for ky in range(3):
            for kx in range(3):
                c3[:C, (ky * 3 + kx) * C:(ky * 3 + kx + 1) * C] = \
                    conv3_slice[:, :, ky, kx].T
        res_wT = np.zeros((128, C), np.float32)
        res_wT[:C] = inp["res_w"].T

        m = {
            "x_loc": np.ascontiguousarray(x_loc),
            "in_wT": in_wT,
            "conv1_w": inp[pre + "conv_w"],
            "conv1_b": inp[pre + "conv_b"].reshape(DI, 1),
            "bigproj_T": np.ascontiguousarray(bigproj.T),
            "bc_wT": np.ascontiguousarray(xproj_w[DTR:].T),
            "dt_b": inp[pre + "dt_b"].reshape(DI, 1),
            "A_mat": -np.exp(inp[pre + "A_log"]),
            "Dvec": inp[pre + "D"].reshape(DI, 1),
            "out_wT": inp[pre + "out_w"].T.astype(ml_dtypes.bfloat16),
            "conv3_w": c3.astype(ml_dtypes.bfloat16),
            "conv3_b": (inp["conv_b"] if d == 0
                        else np.zeros_like(inp["conv_b"])).reshape(C, 1),
            "res_wT": res_wT,
            "res_b": inp["res_b"].reshape(C, 1),
            "bn_g": inp["bn_gamma"].reshape(C, 1),
            "bn_b": inp["bn_beta"].reshape(C, 1),
        }
        maps.append(m)
    return maps


def _run(inputs, trace=False):
    nc = _get_nc()
    maps = _prep_in_maps(inputs)
    res = bass_utils.run_bass_kernel_spmd(
        nc, maps, core_ids=list(range(NCORE)), trace=trace)
    out = np.stack([res.results[2 * b]["out"].reshape(C, H, W)
                    for b in range(B)])
    return out, res


def kernel(**inputs) -> np.ndarray:
    out, _ = _run(inputs, trace=False)
    return out


# revision 29
# speedup vs baseline: 1.5031x; 1.0546x over previous
"""BiMamba block kernel for 8 Trainium2 NeuronCores.

Sharding: core = 2*sample + direction (4 samples x 2 scan directions).
Each core runs the full mamba for its (sample, direction): input/gate
projections (PE), causal depthwise conv (PE diag-matmul), dt softplus
(ACT), then 16 selective-scan states via the DVE tensor_tensor_scan
instruction, with exp(dt*A) on ACT, the dt*x*B / C*h products on DVE in
bf16, and the sum over states accumulated in PSUM by identity matmuls.
The sequence is processed in two halves so the tail of half 0 (gating,
out-projection, partial 3x3 conv) overlaps the scan of half 1.  The
block tail (3x3 conv partials, BatchNorm batch stats, residual,
LeakyReLU) uses a pair AllReduce plus an 8-core stats AllReduce.
"""
import os
import sys

for _p in ("/opt/trn_rl_repo", "/root/.axon_site/_ro/trn_rl_repo"):
    if os.path.isdir(_p):
        if _p not in sys.path:
            sys.path.insert(0, _p)
        break

import ml_dtypes
import numpy as np

# The agent image's antenv lacks axon_hooks; inject it so trace=True can
# capture NTFF profiles (used by test.py for HW timing, not for grading).
try:
    import antenv.axon_hooks  # noqa: F401
except ImportError:
    try:
        import types as _types

        from trn_agent_boot.trn_boot import _ntff_profile_via_ctypes

        _hook = _ntff_profile_via_ctypes("/opt/axon/libaxon_pjrt.so")
        _m = _types.ModuleType("antenv.axon_hooks")
        _m.get_axon_ntff_profile_hook = lambda: _hook
        _m.set_axon_ntff_profile_hook = lambda h: None
        sys.modules["antenv.axon_hooks"] = _m
    except Exception:
        pass

import concourse.bass as bass
import concourse.mybir as mybir
from concourse import bacc
from concourse import bass_utils
from concourse.masks import make_identity
from concourse.tile import TileContext

F32 = mybir.dt.float32
BF16 = mybir.dt.bfloat16
AF = mybir.ActivationFunctionType
OP = mybir.AluOpType

B, C, H, W = 4, 64, 64, 64
L = H * W          # 4096
DI = 128           # d_inner
DS = 16            # d_state
DTR = 4            # dt_rank
DCONV = 4
NCORE = 8
CH = 512           # matmul free-dim chunk
NCH = L // CH      # 8
HALF = L // 2      # scan chunk length
NHALF = 2
RPC = CH // W      # output rows per chunk (8)


def _build():
    nc = bacc.Bacc(target_bir_lowering=False, debug=False, num_devices=NCORE)

    def din(name, shape, dtype=F32):
        return nc.dram_tensor(name, shape, dtype, kind="ExternalInput")

    F32R = mybir.dt.float32r
    x_loc = din("x_loc", [C, L], F32R)
    in_wT = din("in_wT", [128, 2 * DI], F32R)  # padded K (rows 64:128 zero)
    conv1_w = din("conv1_w", [DI, DCONV])
    conv1_b = din("conv1_b", [DI, 1])
    bigproj_T = din("bigproj_T", [DI, DI], F32R)  # (dt_w @ xproj_w[:4]).T
    bc_wT = din("bc_wT", [DI, 2 * DS], F32R)  # xproj_w[4:36].T
    dt_b = din("dt_b", [DI, 1])
    A_mat = din("A_mat", [DI, DS])
    Dvec = din("Dvec", [DI, 1])
    out_wT = din("out_wT", [DI, C], BF16)
    conv3_w = din("conv3_w", [128, 9 * C], BF16)  # tap-major lhsT, padded K
    conv3_b = din("conv3_b", [C, 1])
    res_wT = din("res_wT", [128, C], F32R)    # padded K
    res_b = din("res_b", [C, 1])
    bn_g = din("bn_g", [C, 1])
    bn_b = din("bn_b", [C, 1])

    out_d = nc.dram_tensor("out", [C, L], F32, kind="ExternalOutput")

    with TileContext(nc) as tc:
        with tc.tile_pool(name="pers", bufs=1) as pers:
            # ---- small persistent tiles (params) ----
            p_in_wT = pers.tile([128, 2 * DI], F32R)
            p_c1w = pers.tile([DI, DCONV], F32)
            p_c1b = pers.tile([DI, 1], F32)
            p_bigT = pers.tile([DI, DI], F32R)
            p_bcwT = pers.tile([DI, 2 * DS], F32R)
            p_dtb = pers.tile([DI, 1], F32)
            p_A = pers.tile([DI, DS], F32)
            p_D = pers.tile([DI, 1], F32)
            p_owT = pers.tile([DI, C], BF16)
            p_c3w = pers.tile([128, 9 * C], BF16)
            p_c3b = pers.tile([C, 1], F32)
            p_rwT = pers.tile([128, C], F32R)
            p_rb = pers.tile([C, 1], F32)
            p_bng = pers.tile([C, 1], F32)
            p_bnb = pers.tile([C, 1], F32)
            for t, d in ((p_in_wT, in_wT), (p_c1w, conv1_w), (p_c1b, conv1_b),
                         (p_bigT, bigproj_T), (p_bcwT, bc_wT), (p_dtb, dt_b),
                         (p_A, A_mat), (p_D, Dvec), (p_owT, out_wT),
                         (p_c3w, conv3_w), (p_c3b, conv3_b), (p_rwT, res_wT),
                         (p_rb, res_b), (p_bng, bn_g), (p_bnb, bn_b)):
                nc.sync.dma_start(t[:], d[:])

            ident = pers.tile([128, 128], F32)
            make_identity(nc, ident[:])
            ident_g = pers.tile([128, 128], BF16)
            nc.vector.tensor_copy(ident_g[:], ident[:])
            diag_c1 = [pers.tile([128, 128], F32R, tag=f"dgc{k}", name=f"dgc{k}")
                       for k in range(DCONV)]
            for k in range(DCONV):
                nc.vector.tensor_scalar_mul(diag_c1[k][:], ident[:],
                                            p_c1w[:, k:k + 1])

            # B/C rows (later broadcast per state): [32, L]
            p_bc = pers.tile([2 * DS, L], BF16)
            # DRAM staging copy (DMA partition-broadcast needs a DRAM source)
            bc_dram = nc.dram_tensor("bc_stage", [2 * DS, L], BF16)
            y_gated = pers.tile([DI, L], BF16)

            with tc.tile_pool(name="smid", bufs=1) as smid:
                z_sil = smid.tile([DI, L], BF16)
                dtv = smid.tile([DI, L], F32)
                dtxc_bf = smid.tile([DI, L], BF16)
                xcd = smid.tile([DI, L], BF16)
                carry = smid.tile([DI, DS], F32)

                with tc.tile_pool(name="ph12", bufs=1) as p12, \
                     tc.tile_pool(name="psA", bufs=3, space="PSUM") as psA:
                    x_sb = p12.tile([128, L], F32R)
                    nc.gpsimd.memset(x_sb[64:128, :].bitcast(F32), 0.0)
                    nc.sync.dma_start(x_sb[0:64, :], x_loc[:])
                    xi_pad = p12.tile([DI, 3 + L], F32R)
                    nc.gpsimd.memset(xi_pad[:, 0:3].bitcast(F32), 0.0)
                    xc = p12.tile([DI, L], F32R)

                    # phase 1: xz projection + silu(z); residual projection
                    for c in range(NCH):
                        sl = slice(c * CH, (c + 1) * CH)
                        ps = psA.tile([128, CH], F32, tag="ps")
                        nc.tensor.matmul(ps[:DI], p_in_wT[:, 0:DI],
                                         x_sb[:, sl], start=True, stop=True)
                        nc.scalar.copy(xi_pad[:, 3 + c * CH:3 + (c + 1) * CH],
                                       ps[:DI])
                        ps2 = psA.tile([128, CH], F32, tag="ps")
                        nc.tensor.matmul(ps2[:DI], p_in_wT[:, DI:2 * DI],
                                         x_sb[:, sl], start=True, stop=True)
                        nc.scalar.activation(z_sil[:, sl], ps2[:DI], AF.Silu)

                    # phase 2: causal depthwise conv1d + silu
                    for c in range(NCH):
                        sl = slice(c * CH, (c + 1) * CH)
                        ps = psA.tile([128, CH], F32, tag="ps")
                        for k in range(DCONV):
                            nc.tensor.matmul(
                                ps[:DI], diag_c1[k][:],
                                xi_pad[:, c * CH + k:c * CH + k + CH],
                                start=(k == 0), stop=(k == DCONV - 1))
                        nc.scalar.activation(xc[:, sl], ps[:DI], AF.Silu,
                                             bias=p_c1b[:, 0:1])

                    # phase 3: dt pre-activation + B/C projection
                    # (exp batch, then ln batch, to avoid ACT table reloads)
                    xc_f = xc[:].bitcast(F32)
                    for c in range(NCH):
                        sl = slice(c * CH, (c + 1) * CH)
                        ps = psA.tile([128, CH], F32, tag="ps")
                        nc.tensor.matmul(ps[:DI], p_bigT[:], xc[:, sl],
                                         start=True, stop=True)
                        # softplus = ln(1 + exp(.))
                        nc.scalar.activation(dtv[:, sl], ps[:DI], AF.Exp,
                                             bias=p_dtb[:, 0:1])
                        ps2 = psA.tile([128, CH], F32, tag="ps")
                        nc.tensor.matmul(ps2[:2 * DS], p_bcwT[:], xc[:, sl],
                                         start=True, stop=True)
                        nc.scalar.copy(p_bc[:, sl], ps2[:2 * DS])
                    for hf2 in range(NHALF):
                        hsl2 = slice(hf2 * HALF, (hf2 + 1) * HALF)
                        nc.scalar.activation(dtv[:, hsl2], dtv[:, hsl2],
                                             AF.Ln, bias=1.0)
                        # phase 4 per half: dt*xc (bf16) and xc*D (bf16)
                        nc.vector.tensor_mul(dtxc_bf[:, hsl2], dtv[:, hsl2],
                                             xc_f[:, hsl2])
                        nc.scalar.activation(xcd[:, hsl2], xc_f[:, hsl2],
                                             AF.Copy, scale=p_D[:, 0:1])
                        nc.sync.dma_start(bc_dram[:, hsl2], p_bc[:, hsl2])

                # ---- selective scan + overlapped tail ----
                with tc.tile_pool(name="pp", bufs=8, space="PSUM") as pp, \
                     tc.tile_pool(name="sl_a", bufs=3) as pla, \
                     tc.tile_pool(name="sl_b", bufs=2) as plb, \
                     tc.tile_pool(name="sl_x", bufs=2) as plx, \
                     tc.tile_pool(name="sl_h", bufs=2) as plh, \
                     tc.tile_pool(name="sl_c", bufs=2) as plc, \
                     tc.tile_pool(name="sl_g", bufs=2) as plg, \
                     tc.tile_pool(name="tail", bufs=1) as tl, \
                     tc.tile_pool(name="dram", bufs=1, space="DRAM") as dr:
                    ympad = tl.tile([128, H + 2, W + 2], BF16)
                    res_sb = tl.tile([C, L], F32)
                    x2 = tl.tile([128, L], F32R, tag="xbn")
                    nc.gpsimd.memset(x2[64:128, :].bitcast(F32), 0.0)
                    nc.sync.dma_start(x2[0:64, :], x_loc[:])
                    for c in range(NCH):
                        slr = slice(c * CH, (c + 1) * CH)
                        psr = pp.tile([128, CH], F32, tag="pp", name=f"rs{c}")
                        nc.tensor.matmul(psr[:C], p_rwT[:], x2[:, slr],
                                         start=True, stop=True)
                        nc.scalar.activation(res_sb[:, slr], psr[:C],
                                             AF.Identity, bias=p_rb[:, 0:1])
                    nc.gpsimd.memset(ympad[:], 0.0)
                    conv_part = tl.tile([C, L], F32)
                    PAIRS = [[0, 1], [2, 3], [4, 5], [6, 7]]
                    cc_inA = dr.tile([C, 3 * CH], F32)
                    cc_inB = dr.tile([C, L - 3 * CH], F32)
                    cc_outA = dr.tile([C, 3 * CH], F32)
                    cc_outB = dr.tile([C, L - 3 * CH], F32)

                    def conv3_chunk(c):
                        ps = pp.tile([128, CH], F32, tag="pp",
                                     name=f"cv{c}")
                        ps3 = ps[:C].rearrange("p (r w) -> p r w", w=W)
                        r0 = c * RPC
                        n = 0
                        for ky in range(3):
                            for kx in range(3):
                                nc.tensor.matmul(
                                    ps3[:],
                                    p_c3w[:, (ky * 3 + kx) * C:
                                          (ky * 3 + kx + 1) * C],
                                    ympad[:, r0 + ky:r0 + ky + RPC, kx:kx + W],
                                    start=(n == 0), stop=(n == 8))
                                n += 1
                        nc.scalar.activation(conv_part[:, c * CH:(c + 1) * CH],
                                             ps3.rearrange("p r w -> p (r w)"),
                                             AF.Identity, bias=p_c3b[:, 0:1])

                    for hf in range(NHALF):
                        y_ps = []
                        for j in range(NCH // 2):
                            cix = hf * (NCH // 2) + j
                            yp = pp.tile([128, CH], F32, tag="pp",
                                         name=f"y{cix}")
                            nc.tensor.matmul(
                                yp[:DI], ident_g[:],
                                xcd[:, cix * CH:(cix + 1) * CH],
                                start=True, stop=False)
                            y_ps.append(yp)

                        hsl = slice(hf * HALF, (hf + 1) * HALF)
                        for s in range(DS):
                            da = pla.tile([DI, HALF], F32, tag="da")
                            nc.scalar.activation(da[:], dtv[:, hsl], AF.Exp,
                                                 scale=p_A[:, s:s + 1])
                            bbc = plb.tile([DI, HALF], BF16, tag="bbc")
                            nc.sync.dma_start(
                                bbc[:],
                                bc_dram[s:s + 1, hsl].to_broadcast((DI, HALF)))
                            dbx = plx.tile([DI, HALF], BF16, tag="dbx")
                            nc.vector.tensor_mul(dbx[:], dtxc_bf[:, hsl],
                                                 bbc[:])
                            h = plh.tile([DI, HALF], BF16, tag="h")
                            init = 0.0 if hf == 0 else carry[:, s:s + 1]
                            nc.vector.tensor_tensor_scan(
                                h[:], da[:], dbx[:], init,
                                op0=OP.mult, op1=OP.add)
                            if hf == 0:
                                nc.vector.tensor_copy(carry[:, s:s + 1],
                                                      h[:, HALF - 1:HALF])
                            cbc = plc.tile([DI, HALF], BF16, tag="cbc")
                            nc.sync.dma_start(
                                cbc[:],
                                bc_dram[DS + s:DS + s + 1,
                                        hsl].to_broadcast((DI, HALF)))
                            g = plg.tile([DI, HALF], BF16, tag="g")
                            nc.vector.tensor_mul(g[:], h[:], cbc[:])
                            for j in range(NCH // 2):
                                nc.tensor.matmul(
                                    y_ps[j][:DI], ident_g[:],
                                    g[:, j * CH:(j + 1) * CH],
                                    start=False, stop=(s == DS - 1))

                        # gating + out-projection + padded spatial write
                        for j in range(NCH // 2):
                            cix = hf * (NCH // 2) + j
                            sl = slice(cix * CH, (cix + 1) * CH)
                            nc.vector.tensor_mul(y_gated[:, sl],
                                                 y_ps[j][:DI], z_sil[:, sl])
                            po = pp.tile([128, CH], F32, tag="pp",
                                         name=f"po{cix}")
                            nc.tensor.matmul(po[:C], p_owT[:],
                                             y_gated[:, sl],
                                             start=True, stop=True)
                            r0 = cix * RPC
                            nc.scalar.copy(
                                ympad[0:C, 1 + r0:1 + r0 + RPC, 1:1 + W],
                                po[:C].rearrange("p (r w) -> p r w", w=W))

                        # 3x3 conv on rows whose inputs are complete,
                        # then kick off the pair AllReduce of that span
                        if hf == 0:
                            for c in range(0, 3):
                                conv3_chunk(c)
                            nc.sync.dma_start(cc_inA[:],
                                              conv_part[:, 0:3 * CH])
                            nc.gpsimd.collective_compute(
                                "AllReduce", OP.add,
                                replica_groups=PAIRS,
                                ins=[cc_inA[:].opt()],
                                outs=[cc_outA[:].opt()])
                        else:
                            for c in range(3, NCH):
                                conv3_chunk(c)
                            nc.sync.dma_start(cc_inB[:],
                                              conv_part[:, 3 * CH:L])
                            nc.gpsimd.collective_compute(
                                "AllReduce", OP.add,
                                replica_groups=PAIRS,
                                ins=[cc_inB[:].opt()],
                                outs=[cc_outB[:].opt()])

                    conv_full = tl.tile([C, L], F32)
                    nc.sync.dma_start(conv_full[:, 0:3 * CH], cc_outA[:])
                    nc.sync.dma_start(conv_full[:, 3 * CH:L], cc_outB[:])

                    # BN statistics per span (span A overlaps the second
                    # half of the scan), then 8-core AllReduce
                    stats2 = tl.tile([C, 2, 2], F32)
                    stats = tl.tile([C, 2], F32)
                    for si, (lo, hi) in enumerate(((0, 3 * CH), (3 * CH, L))):
                        nc.vector.tensor_reduce(stats2[:, si, 0:1],
                                                conv_full[:, lo:hi],
                                                axis=mybir.AxisListType.X,
                                                op=OP.add)
                        # square scratch overwrites conv_part (dead after
                        # the collective input DMAs); only accum_out is used
                        nc.scalar.activation(conv_part[:, lo:hi],
                                             conv_full[:, lo:hi], AF.Square,
                                             accum_out=stats2[:, si, 1:2])
                    nc.vector.tensor_add(stats[:], stats2[:, 0], stats2[:, 1])
                    st_in = dr.tile([C, 2], F32)
                    st_out = nc.dram_tensor("st_out", [C, 2], F32,
                                            addr_space="Shared")
                    nc.sync.dma_start(st_in[:], stats[:])
                    nc.gpsimd.collective_compute(
                        "AllReduce", OP.add,
                        replica_groups=[[0, 1, 2, 3, 4, 5, 6, 7]],
                        ins=[st_in[:].opt()], outs=[st_out[:].opt()])
                    stot = tl.tile([C, 2], F32)
                    nc.sync.dma_start(stot[:], st_out[:])

                    inv_n = 1.0 / (2.0 * B * L)
                    mean = tl.tile([C, 1], F32)
                    var = tl.tile([C, 1], F32)
                    tmp = tl.tile([C, 1], F32)
                    nc.vector.tensor_scalar_mul(mean[:], stot[:, 0:1], inv_n)
                    nc.vector.tensor_scalar_mul(var[:], stot[:, 1:2], inv_n)
                    nc.vector.tensor_mul(tmp[:], mean[:], mean[:])
                    nc.vector.tensor_sub(var[:], var[:], tmp[:])
                    # invstd = 1/sqrt(var + eps)
                    nc.vector.tensor_scalar_add(var[:], var[:], 1e-5)
                    nc.scalar.activation(tmp[:], var[:], AF.Sqrt)
                    nc.vector.reciprocal(tmp[:], tmp[:])
                    scal = tl.tile([C, 1], F32)
                    shft = tl.tile([C, 1], F32)
                    nc.vector.tensor_mul(scal[:], p_bng[:], tmp[:])
                    nc.vector.tensor_mul(tmp[:], mean[:], scal[:])
                    nc.vector.tensor_sub(shft[:], p_bnb[:], tmp[:])

                    # bn + residual + leaky relu:
                    #   out = prelu(conv*scal + res + shft)
                    bn1 = tl.tile([128, L], F32, tag="xbn", name="bn1")[:C]
                    nc.vector.scalar_tensor_tensor(bn1[:], conv_full[:],
                                                   scal[:, 0:1], res_sb[:],
                                                   op0=OP.mult, op1=OP.add)
                    nc.scalar.activation(bn1[:], bn1[:], AF.Prelu,
                                         alpha=0.01, bias=shft[:, 0:1])
                    nc.sync.dma_start(out_d[:], bn1[:])

    nc.compile()
    return nc


_NC = None


def _get_nc():
    global _NC
    if _NC is None:
        _NC = _build()
    return _NC


def _prep_in_maps(inp):
    inp = {k: np.asarray(v, dtype=np.float32) for k, v in inp.items()}
    x = inp["x"]  # (4, 64, 64, 64)
    maps = []
    for core in range(NCORE):
        b, d = core // 2, core % 2
        pre = "m1_" if d == 0 else "m2_"
        in_w = inp[pre + "in_w"]          # (256, 64)
        xproj_w = inp[pre + "xproj_w"]    # (36, 128)
        dt_w = inp[pre + "dt_w"]          # (128, 4)

        x_loc = x[b].reshape(C, L)
        if d == 1:
            x_loc = x_loc[:, ::-1]

        in_wT = np.zeros((128, 2 * DI), np.float32)
        in_wT[:C] = in_w.T
        bigproj = dt_w @ xproj_w[:DTR]    # (128, 128)
        conv3_slice = inp["conv_w"][:, d * C:(d + 1) * C]  # (64,64,3,3)
        c3 = np.zeros((128, 9 * C), np.float32)
        for ky in range(3):
            for kx in range(3):
                c3[:C, (ky * 3 + kx) * C:(ky * 3 + kx + 1) * C] = \
                    conv3_slice[:, :, ky, kx].T
        res_wT = np.zeros((128, C), np.float32)
        res_wT[:C] = inp["res_w"].T

        m = {
            "x_loc": np.ascontiguousarray(x_loc),
            "in_wT": in_wT,
            "conv1_w": inp[pre + "conv_w"],
            "conv1_b": inp[pre + "conv_b"].reshape(DI, 1),
            "bigproj_T": np.ascontiguousarray(bigproj.T),
            "bc_wT": np.ascontiguousarray(xproj_w[DTR:].T),
            "dt_b": inp[pre + "dt_b"].reshape(DI, 1),
            "A_mat": -np.exp(inp[pre + "A_log"]),
            "Dvec": inp[pre + "D"].reshape(DI, 1),
            "out_wT": inp[pre + "out_w"].T.astype(ml_dtypes.bfloat16),
            "conv3_w": c3.astype(ml_dtypes.bfloat16),
            "conv3_b": (inp["conv_b"] if d == 0
                        else np.zeros_like(inp["conv_b"])).reshape(C, 1),
            "res_wT": res_wT,
            "res_b": inp["res_b"].reshape(C, 1),
            "bn_g": inp["bn_gamma"].reshape(C, 1),
            "bn_b": inp["bn_beta"].reshape(C, 1),
        }
        maps.append(m)
    return maps


def _run(inputs, trace=False):
    nc = _get_nc()
    maps = _prep_in_maps(inputs)
    res = bass_utils.run_bass_kernel_spmd(
        nc, maps, core_ids=list(range(NCORE)), trace=trace)
    out = np.stack([res.results[2 * b]["out"].reshape(C, H, W)
                    for b in range(B)])
    return out, res


def kernel(**inputs) -> np.ndarray:
    out, _ = _run(inputs, trace=False)
    return out
